# revision 1
# baseline (speedup 1.0000x reference)
"""CliffordSpectralConv2d on 8 trn2 NeuronCores.

Math: the reference is, per sample b and "dual pair" (d1 = x0 + i*x3,
d2 = x1 + i*x2):
    Y_d   = A @ X_d @ A^T          (crop-DFT, A = F256[rows 0:32 + 224:256])
    OD1   = sum_c W1*Y1 + W2*conj(Y2)   (positionwise over the 64x64 modes)
    OD2   = sum_c W1*Y2 + W2*conj(Y1)   (W1 = w0 + i*w3, W2 = w1 + i*w2)
    out_d = (1/65536) A^H @ OD_d @ conj(A)
with out components (re(o1), re(o2), im(o2), im(o1)).

Sharding (8 cores, one NEFF, SPMD):
  core k = (b = k%4, half = k//4)
  phase F: forward crop-DFT for x[b, 16*half:16*half+16] (32 complex ch)
  AllToAll #1 (1 MB/rank): reshard Y by mode-row slices
  phase M: positionwise mode-mix as 512 (K=128 -> M=128, N=4) matmuls
           with host-prebuilt per-position block matrices (bf16)
  AllToAll #2 (1 MB/rank): reshard OD by (b, out-channel-half)
  phase I: inverse DFT for 16 output channels, interleave components,
           write out[b, 16*half:16*half+16] (contiguous)
"""

import numpy as np
import ml_dtypes

import concourse.bass as bass
import concourse.mybir as mybir
import concourse.tile as tile
from concourse import bacc
from concourse.bass_utils import run_bass_kernel_spmd

NCORES = 8
B, CIN, COUT, H, W = 4, 32, 32, 256, 256
M = 32            # modes per corner
M2 = 64           # 2*M
CH = 16           # channels per core (forward)
OH = 16           # out channels per core (inverse)
ROWS = 8          # mode rows per core (mix)
POS = ROWS * M2   # positions per core (512)

FP32 = mybir.dt.float32
FP32R = mybir.dt.float32r
BF16 = mybir.dt.bfloat16

# dtype knobs (accuracy vs speed)
MIX_DT = BF16     # mode-mix matmul dtype
S2_DT = FP32      # stage-2 / inverse fp32r experiments flip these
I2_DT = FP32R     # inverse second matmul (full rate at N=256)

_prep_cache = {}
_result_cache = {}


def _dft_mats():
    k = np.arange(H)
    sel = np.concatenate([np.arange(M), np.arange(H - M, H)])
    F = np.exp(-2j * np.pi * np.outer(k, k) / H)
    A = F[sel, :]
    return A.real.astype(np.float32).copy(), A.imag.astype(np.float32).copy()


def _host_consts():
    Ar, Ai = _dft_mats()  # (64, 256)
    # rx[ck, 0] = [Ar_chunk^T | Ai_chunk^T]; rx[ck, 1] = [-Ai_chunk^T | Ar_chunk^T]
    rx = np.zeros((2, 2, 128, 128), np.float32)
    for ck in range(2):
        ArT = Ar[:, ck * 128:(ck + 1) * 128].T  # (128, 64)
        AiT = Ai[:, ck * 128:(ck + 1) * 128].T
        rx[ck, 0, :, :64], rx[ck, 0, :, 64:] = ArT, AiT
        rx[ck, 1, :, :64], rx[ck, 1, :, 64:] = -AiT, ArT
    # ia[hb, 0] = [Ar_chunk; Ai_chunk] rows; ia[hb, 1] = [-Ai_chunk; Ar_chunk]
    ia = np.zeros((2, 2, 128, 128), np.float32)
    for hb in range(2):
        Arc = Ar[:, hb * 128:(hb + 1) * 128]  # (64, 128)
        Aic = Ai[:, hb * 128:(hb + 1) * 128]
        ia[hb, 0, :64], ia[hb, 0, 64:] = Arc, Aic
        ia[hb, 1, :64], ia[hb, 1, 64:] = -Aic, Arc
    # ib[0] = [Ar; Ai]/65536 ; ib[1] = [-Ai; Ar]/65536   (128, 256)
    s = 1.0 / float(H * W)
    ib = np.zeros((2, 128, 256), np.float32)
    ib[0, :64], ib[0, 64:] = Ar * s, Ai * s
    ib[1, :64], ib[1, 64:] = -Ai * s, Ar * s
    ident = np.eye(128, dtype=np.float32)
    return rx, ia, ib, ident


def _build_kmat(weights):
    """(4096, 128, 128) per-position mix matrices, lhsT layout [i, o].

    i-blocks (K): [Y1r, Y1i, Y2r, Y2i] x 32c; o-blocks (M): [OD1r, OD1i,
    OD2r, OD2i] x 32o.  Entry = sign * w_s[o, c, pos] per the complex
    product grid; block [i=b*32+c, o=g*32+ol] = sign(b,g) * w_{s(b,g)}[ol, c].
    """
    w = np.asarray(weights, np.float32)  # (4, 32, 32, 64, 64)
    W1r, W1i, W2r, W2i = w[0], w[3], w[1], w[2]
    grid = [
        [(W1r, 1.0), (W1i, 1.0), (W2r, 1.0), (W2i, 1.0)],
        [(W1i, -1.0), (W1r, 1.0), (W2i, 1.0), (W2r, -1.0)],
        [(W2r, 1.0), (W2i, 1.0), (W1r, 1.0), (W1i, 1.0)],
        [(W2i, 1.0), (W2r, -1.0), (W1i, -1.0), (W1r, 1.0)],
    ]
    np_dt = mybir.dt.np(MIX_DT)
    km = np.zeros((4096, 128, 128), np_dt)
    for bi in range(4):
        for gi in range(4):
            wm, sign = grid[bi][gi]
            # [o, c, m1, m2] -> [pos, c, o]
            blk = (sign * wm).transpose(2, 3, 1, 0).reshape(4096, 32, 32)
            km[:, bi * 32:(bi + 1) * 32, gi * 32:(gi + 1) * 32] = blk.astype(np_dt)
    return km


def _emit(nc, dbg=False):
    """Emit the SPMD program (same for every core; data differs)."""
    xs = nc.dram_tensor("xs", [CH, H, W, 4], FP32, kind="ExternalInput").ap()
    km = nc.dram_tensor("km", [POS, 128, 128], MIX_DT, kind="ExternalInput").ap()
    rx = nc.dram_tensor("rx", [2, 2, 128, 128], FP32, kind="ExternalInput").ap()
    ia = nc.dram_tensor("ia", [2, 2, 128, 128], FP32, kind="ExternalInput").ap()
    ib = nc.dram_tensor("ib", [2, 128, 256], I2_DT, kind="ExternalInput").ap()
    ident = nc.dram_tensor("ident", [128, 128], FP32, kind="ExternalInput").ap()
    oout = nc.dram_tensor("o", [OH, H, W, 4], FP32, kind="ExternalOutput").ap()
    if dbg:
        ydbg = nc.dram_tensor("ydbg", [64, 2, 2, CH, M2], FP32, kind="ExternalOutput").ap()
        adbg = nc.dram_tensor("adbg", [8, ROWS, 2, 2, CH, M2], FP32, kind="ExternalOutput").ap()
        bdbg = nc.dram_tensor("bdbg", [8, 4, OH, 8, M2], FP32, kind="ExternalOutput").ap()
        sdbg = nc.dram_tensor("sdbg", [4, 128, 512], FP32, kind="ExternalOutput").ap()
        yhdbg = nc.dram_tensor("yhdbg", [ROWS, 128, 256], FP32, kind="ExternalOutput").ap()
        bsdbg = nc.dram_tensor("bsdbg", [8, 4, OH, 4, 128], FP32, kind="ExternalOutput").ap()

    with tile.TileContext(nc) as tc:
        with (
            tc.tile_pool(name="consts", bufs=1) as cpool,
            tc.tile_pool(name="dram", bufs=1, space="DRAM") as dpool,
        ):
            # resident constants (partition dim must be first -> one tile each)
            rxs, ias = {}, {}
            for ck in range(2):
                for j in range(2):
                    t = cpool.tile([128, 128], FP32, name=f"rxs{ck}{j}")
                    nc.sync.dma_start(out=t[:], in_=rx[ck, j])
                    rxs[ck, j] = t
                    t2 = cpool.tile([128, 128], FP32, name=f"ias{ck}{j}")
                    nc.sync.dma_start(out=t2[:], in_=ia[ck, j])
                    ias[ck, j] = t2
            ibs = {}
            for j in range(2):
                t = cpool.tile([128, 256], I2_DT, name=f"ibs{j}")
                nc.sync.dma_start(out=t[:], in_=ib[j])
                ibs[j] = t
            ids = cpool.tile([128, 128], FP32, name="ids")
            nc.sync.dma_start(out=ids[:], in_=ident[:])

            # collective buffers
            ybuf = dpool.tile([64, 2, 2, CH, M2], FP32, name="ybuf")
            arecv = dpool.tile([8, ROWS, 2, 2, CH, M2], FP32, name="arecv")
            bsend = dpool.tile([8, 4, OH, 4, 128], FP32, name="bsend")
            brecv = dpool.tile([8, 4, OH, 8, M2], FP32, name="brecv")

            # ---------------- phase F: forward crop-DFT ----------------
            with (
                tc.tile_pool(name="fsb", bufs=3) as fsb,
                tc.tile_pool(name="ftt", bufs=2) as ftt,
                tc.tile_pool(name="fps", bufs=2, space="PSUM") as fps,
            ):
                for c in range(CH):
                    xt0 = fsb.tile([128, W * 4], FP32, tag="xt0")
                    nc.sync.dma_start(
                        out=xt0[:], in_=xs[c, 0:128].rearrange("h w k -> h (w k)"))
                    xt1 = fsb.tile([128, W * 4], FP32, tag="xt1")
                    nc.sync.dma_start(
                        out=xt1[:], in_=xs[c, 128:256].rearrange("h w k -> h (w k)"))
                    xv = [xt0.rearrange("h (w k) -> h k w", k=4),
                          xt1.rearrange("h (w k) -> h k w", k=4)]
                    for d in range(2):
                        re_c, im_c = (0, 3) if d == 0 else (1, 2)
                        tts = []
                        for wb in range(2):
                            pt = fps.tile([128, 128], FP32, tag="pt")
                            for hk in range(2):
                                nc.tensor.matmul(
                                    pt[:],
                                    lhsT=xv[hk][:, re_c, wb * 128:(wb + 1) * 128],
                                    rhs=rxs[hk, 0][:],
                                    start=(hk == 0), stop=False)
                                nc.tensor.matmul(
                                    pt[:],
                                    lhsT=xv[hk][:, im_c, wb * 128:(wb + 1) * 128],
                                    rhs=rxs[hk, 1][:],
                                    start=False, stop=(hk == 1))
                            tt = ftt.tile([128, 128], FP32, tag=f"tt{wb}")
                            nc.vector.tensor_copy(tt[:], pt[:])
                            tts.append(tt)
                        py = fps.tile([64, 128], FP32, tag="py")
                        for wb in range(2):
                            nc.tensor.matmul(
                                py[:], lhsT=tts[wb][:, 0:64], rhs=rxs[wb, 0][:],
                                start=(wb == 0), stop=False)
                            nc.tensor.matmul(
                                py[:], lhsT=tts[wb][:, 64:128], rhs=rxs[wb, 1][:],
                                start=False, stop=(wb == 1))
                        sy = ftt.tile([64, 128], FP32, tag="sy")
                        nc.vector.tensor_copy(sy[:], py[:])
                        nc.sync.dma_start(
                            out=ybuf[:, d, :, c, :],
                            in_=sy.rearrange("m (r n) -> m r n", r=2))

            nc.gpsimd.collective_compute(
                "AllToAll", mybir.AluOpType.bypass,
                replica_groups=[list(range(NCORES))],
                ins=[ybuf.rearrange("a b c d e -> a (b c d e)").opt()],
                outs=[arecv.rearrange("a b c d e f -> a (b c d e f)").opt()],
            )
            if dbg:
                nc.sync.dma_start(out=ydbg[:], in_=ybuf[:])
                nc.sync.dma_start(out=adbg[:], in_=arecv[:])

            # ---------------- phase M: mode mix ----------------
            with (
                tc.tile_pool(name="msb", bufs=3) as msb,
                tc.tile_pool(name="mps", bufs=2, space="PSUM") as mps,
            ):
                for half in range(4):  # 2 rows -> 128 positions each
                    pod = mps.tile([128, 512], FP32, tag="pod")
                    for rr in range(2):
                        r = half * 2 + rr
                        yb = msb.tile([128, 256], FP32, tag="yb")
                        for b in range(4):
                            for h in range(2):
                                for d in range(2):
                                    for ri in range(2):
                                        p0 = (d * 2 + ri) * 32 + h * CH
                                        nc.sync.dma_start(
                                            out=yb[p0:p0 + CH,
                                                   b * 64:(b + 1) * 64],
                                            in_=arecv[h * 4 + b, r, d, ri])
                        if dbg:
                            nc.sync.dma_start(out=yhdbg[r], in_=yb[:])
                        ybb = msb.tile([128, 256], MIX_DT, tag="ybb")
                        nc.vector.tensor_copy(ybb[:], yb[:])
                        ybbv = ybb.rearrange("i (b m) -> i b m", b=4)
                        for qb in range(8):  # 8 positions per kt tile
                            kt = msb.tile([128, 8 * 128], MIX_DT, tag="kt")
                            p0 = r * M2 + qb * 8
                            nc.sync.dma_start(
                                out=kt.rearrange("i (p o) -> i p o", p=8),
                                in_=km[p0:p0 + 8].rearrange("p i o -> i p o"))
                            for q in range(8):
                                m2 = qb * 8 + q
                                p4 = (rr * 64 + m2) * 4
                                nc.tensor.matmul(
                                    pod[:, p4:p4 + 4],
                                    lhsT=kt[:, q * 128:(q + 1) * 128],
                                    rhs=ybbv[:, :, m2],
                                    start=True, stop=True)
                    sod = msb.tile([128, 512], FP32, tag="sod")
                    nc.vector.tensor_copy(
                        sod.rearrange("o (b p) -> o b p", b=4),
                        pod.rearrange("o (p b) -> o p b", p=128)
                           .rearrange("o p b -> o b p"))
                    if dbg:
                        nc.sync.dma_start(out=sdbg[half], in_=sod[:])
                    for dst in range(8):
                        bp, ohp = dst % 4, dst // 4
                        for bt in range(4):
                            p0 = bt * 32 + ohp * OH
                            nc.sync.dma_start(
                                out=bsend[dst, bt, :, half, :],
                                in_=sod[p0:p0 + OH, bp * 128:(bp + 1) * 128])

            nc.gpsimd.collective_compute(
                "AllToAll", mybir.AluOpType.bypass,
                replica_groups=[list(range(NCORES))],
                ins=[bsend.rearrange("a b c d e -> a (b c d e)").opt()],
                outs=[brecv.rearrange("a b c d e -> a (b c d e)").opt()],
            )
            if dbg:
                nc.sync.dma_start(out=bdbg[:], in_=brecv[:])
                nc.sync.dma_start(out=bsdbg[:], in_=bsend[:])

            # ---------------- phase I: inverse DFT ----------------
            with (
                tc.tile_pool(name="isb", bufs=3) as isb,
                tc.tile_pool(name="ips", bufs=2, space="PSUM") as ips,
                tc.tile_pool(name="ops", bufs=1, space="PSUM") as ops,
            ):
                for ol in range(OH):
                    pos = []  # psum_o[d][hb]
                    for d in range(2):
                        ods = isb.tile([128, 64], FP32, tag="ods")
                        for u in range(2):
                            for sc in range(8):
                                nc.sync.dma_start(
                                    out=ods[u * 64 + sc * 8:u * 64 + sc * 8 + 8, :],
                                    in_=brecv[sc, 2 * d + u, ol])
                        row = []
                        for hb in range(2):
                            pv = ips.tile([128, 128], FP32, tag="pv")
                            nc.tensor.matmul(pv[:, 0:64], lhsT=ias[hb, 0][:],
                                             rhs=ods[:], start=True, stop=True)
                            nc.tensor.matmul(pv[:, 64:128], lhsT=ias[hb, 1][:],
                                             rhs=ods[:], start=True, stop=True)
                            sv = isb.tile([128, 128], FP32, tag="sv")
                            nc.vector.tensor_copy(sv[:], pv[:])
                            pvt = ips.tile([128, 128], FP32, tag="pvt")
                            nc.tensor.transpose(pvt[:], sv[:], ids[:])
                            svt = isb.tile([128, 128], I2_DT, tag="svt")
                            nc.vector.tensor_copy(svt[:], pvt[:])
                            po = ops.tile([128, 512], FP32, tag=f"po{d}{hb}")
                            nc.tensor.matmul(po[:, 0:256], lhsT=svt[:],
                                             rhs=ibs[0][:], start=True, stop=True)
                            nc.tensor.matmul(po[:, 256:512], lhsT=svt[:],
                                             rhs=ibs[1][:], start=True, stop=True)
                            row.append(po)
                        pos.append(row)
                    for hb in range(2):
                        so = isb.tile([128, W * 4], FP32, tag="so")
                        sov = so.rearrange("p (w k) -> p w k", k=4)
                        nc.vector.tensor_copy(sov[:, :, 0], pos[0][hb][:, 0:256])
                        nc.vector.tensor_copy(sov[:, :, 3], pos[0][hb][:, 256:512])
                        nc.vector.tensor_copy(sov[:, :, 1], pos[1][hb][:, 0:256])
                        nc.vector.tensor_copy(sov[:, :, 2], pos[1][hb][:, 256:512])
                        nc.sync.dma_start(
                            out=oout[ol, hb * 128:(hb + 1) * 128].rearrange(
                                "h w k -> h (w k)"),
                            in_=so[:])
    return nc


LAST_EXEC_NS = None
LAST_RUN_WALL_NS = None


def kernel(x, weights, _dbg=False, _trace=False):
    x = np.ascontiguousarray(np.asarray(x, np.float32))
    weights = np.asarray(weights, np.float32)

    rxc, iac, ibc, identc = _host_consts()
    kmat = _build_kmat(weights)
    ib_np = ibc.astype(mybir.dt.np(I2_DT))

    nc = _prep_cache.get(_dbg)
    if nc is None:
        nc = bacc.Bacc("TRN2", target_bir_lowering=False, debug=False,
                       enable_asserts=False, num_devices=NCORES)
        _emit(nc, dbg=_dbg)
        nc.compile()
        _prep_cache[_dbg] = nc

    in_maps = []
    for k in range(NCORES):
        b, half = k % 4, k // 4
        in_maps.append({
            "xs": np.ascontiguousarray(x[b, half * CH:(half + 1) * CH]),
            "km": np.ascontiguousarray(
                kmat.reshape(64, M2, 128, 128)[ROWS * k:ROWS * (k + 1)]
                .reshape(POS, 128, 128)),
            "rx": rxc, "ia": iac, "ib": ib_np, "ident": identc,
        })

    global LAST_EXEC_NS, LAST_RUN_WALL_NS
    kw = {}
    if _trace:
        kw = dict(trace=True, trace_cores=list(range(NCORES)))
    import time as _time
    _t0 = _time.perf_counter()
    res = run_bass_kernel_spmd(nc, in_maps, core_ids=list(range(NCORES)), **kw)
    LAST_RUN_WALL_NS = int((_time.perf_counter() - _t0) * 1e9)
    if res.exec_time_ns is not None:
        LAST_EXEC_NS = res.exec_time_ns
    out = np.empty((B, COUT, H, W, 4), np.float32)
    for k in range(NCORES):
        b, half = k % 4, k // 4
        out[b, half * OH:(half + 1) * OH] = res.results[k]["o"]
    if _dbg:
        return out, res.results
    return out


if __name__ == "__main__":
    xs = np.random.randn(B, CIN, H, W, 4).astype(np.float32)
    ws = np.random.rand(4, COUT, CIN, M2, M2).astype(np.float32) / (CIN * COUT)
    out = kernel(xs, ws)
    print(out.shape, out.dtype)

